# revision 5
# baseline (speedup 1.0000x reference)
"""Trainium2 Bass kernel for nn_Discriminator (GNN edge discriminator).

Algebraic reduction: with w1=W_edge[:H], w2=W_edge[H:],
  s = 0.5*((src@w1+dst@w2) + (dst@w1+src@w2)) = (src+dst) @ wbar,  wbar=(w1+w2)/2
so each node needs only the scalar g[n] = relu(emb[n]@W+b) @ wbar (+ b_edge/2),
and each edge needs s = g[e0]+g[e1]; out = sigmoid(log(eps)-log1p(-eps)+s).

Sharding: nodes row-sharded 6250/core for the g computation, then AllGather;
edges sharded 200K/core. The [50000] g table is held in SBUF 16-way
mod-interleaved (table[p,w] = g[16w+p%16]); per-edge lookups use POOL
indirect_copy (shared column index per 16-partition group = the 16 candidate
values), then a PE broadcast + DVE masked-select + PE block-diagonal reduce
collapse the candidates, accumulating g[e0]+g[e1] straight into PSUM.
"""
import sys
sys.path.insert(0, '/opt/trn_rl_repo')
import numpy as np

N, IN_DIM, HID = 50000, 256, 64
E = 1_600_000
BIAS = 0.0001
NCORES = 8
EC = E // NCORES            # 200000 edges per core
M = 512                     # indices per group per gather instruction
TILE_E = 8 * M              # 4096 edges per pair-tile
NTILES = 49                 # 49*4096 = 200704
ECP = NTILES * TILE_E       # padded edges per core
NNC = N // NCORES           # 6250 nodes per core
NNCP = 6272                 # padded to 49*128 (12*512 + 128)
TAB_W = 3125                # 50000 / 16

_nc = None
_runner = None


def _build():
    from concourse import bass, bacc, tile, mybir

    f32 = mybir.dt.float32
    nc = bacc.Bacc("TRN2", target_bir_lowering=False, debug=False,
                   num_devices=NCORES)

    embT = nc.dram_tensor("embT", [2, 128, NNCP], f32, kind="ExternalInput")
    e0_d = nc.dram_tensor("e0", [ECP], mybir.dt.int32, kind="ExternalInput")
    e1_d = nc.dram_tensor("e1", [ECP], mybir.dt.int32, kind="ExternalInput")
    noise_d = nc.dram_tensor("noise", [ECP], f32, kind="ExternalInput")
    wemb_d = nc.dram_tensor("wemb", [2, 128, HID], f32, kind="ExternalInput")
    bemb_d = nc.dram_tensor("bemb", [HID, 1], f32, kind="ExternalInput")
    wbar_d = nc.dram_tensor("wbar", [HID, 1], f32, kind="ExternalInput")
    bhalf_d = nc.dram_tensor("bhalf", [1, 1], f32, kind="ExternalInput")
    expand8_d = nc.dram_tensor("expand8", [8, 128], f32, kind="ExternalInput")
    bdiag8_d = nc.dram_tensor("bdiag8", [128, 8], f32, kind="ExternalInput")
    iota16_d = nc.dram_tensor("iota16", [128, 1], f32, kind="ExternalInput")
    out_d = nc.dram_tensor("out", [ECP], f32, kind="ExternalOutput")

    a1, b1 = 2.0 * BIAS - 1.0, 1.0 - BIAS     # eps = a1*n + b1
    a2, b2 = 1.0 - 2.0 * BIAS, BIAS           # 1-eps = a2*n + b2

    with tile.TileContext(nc) as tc:
        with tc.tile_pool(name="const", bufs=1) as cp, \
             tc.tile_pool(name="tab", bufs=1) as tabp, \
             tc.tile_pool(name="dram", bufs=1, space="DRAM") as dram:
            w0 = cp.tile([128, HID], f32, tag="w0")
            w1 = cp.tile([128, HID], f32, tag="w1")
            nc.sync.dma_start(out=w0[:], in_=wemb_d[0])
            nc.sync.dma_start(out=w1[:], in_=wemb_d[1])
            bemb = cp.tile([HID, 1], f32, tag="bemb")
            nc.sync.dma_start(out=bemb[:], in_=bemb_d[:, :])
            wbar = cp.tile([HID, 1], f32, tag="wbar")
            nc.sync.dma_start(out=wbar[:], in_=wbar_d[:, :])
            bhalf = cp.tile([1, 1], f32, tag="bhalf")
            nc.sync.dma_start(out=bhalf[:], in_=bhalf_d[:, :])
            expand8 = cp.tile([8, 128], f32, tag="ex8")
            nc.sync.dma_start(out=expand8[:], in_=expand8_d[:, :])
            bdiag8 = cp.tile([128, 8], f32, tag="bd8")
            nc.sync.dma_start(out=bdiag8[:], in_=bdiag8_d[:, :])
            iota16 = cp.tile([128, 1], f32, tag="io16")
            nc.sync.dma_start(out=iota16[:], in_=iota16_d[:, :])
            a1t = cp.tile([8, 1], f32, tag="a1t"); nc.vector.memset(a1t[:], a1)
            b1t = cp.tile([8, 1], f32, tag="b1t"); nc.vector.memset(b1t[:], b1)
            a2t = cp.tile([8, 1], f32, tag="a2t"); nc.vector.memset(a2t[:], a2)
            b2t = cp.tile([8, 1], f32, tag="b2t"); nc.vector.memset(b2t[:], b2)

            # ---------------- phase 1: per-node scalar g ----------------
            g_sb = cp.tile([1, NNCP], f32, tag="gsb")
            with tc.tile_pool(name="p1", bufs=3) as p1, \
                 tc.tile_pool(name="ps1", bufs=2, space="PSUM") as ps1, \
                 tc.tile_pool(name="ps1g", bufs=2, space="PSUM") as ps1g:
                col = 0
                while col < NNCP:
                    n = min(512, NNCP - col)
                    r0 = p1.tile([128, n], f32, tag="r0")
                    r1 = p1.tile([128, n], f32, tag="r1")
                    nc.sync.dma_start(out=r0[:], in_=embT[0, :, col:col + n])
                    nc.sync.dma_start(out=r1[:], in_=embT[1, :, col:col + n])
                    ph = ps1.tile([HID, n], f32, tag="ph")
                    nc.tensor.matmul(out=ph[:], lhsT=w0[:], rhs=r0[:],
                                     start=True, stop=False)
                    nc.tensor.matmul(out=ph[:], lhsT=w1[:], rhs=r1[:],
                                     start=False, stop=True)
                    hT = p1.tile([HID, n], f32, tag="hT")
                    nc.scalar.activation(out=hT[:], in_=ph[:],
                                         func=mybir.ActivationFunctionType.Relu,
                                         bias=bemb[:, 0:1])
                    pg = ps1g.tile([1, n], f32, tag="pg")
                    nc.tensor.matmul(out=pg[:], lhsT=wbar[:], rhs=hT[:],
                                     start=True, stop=True)
                    nc.scalar.activation(out=g_sb[0:1, col:col + n], in_=pg[:],
                                         func=mybir.ActivationFunctionType.Identity,
                                         bias=bhalf[0:1, 0:1])
                    col += n

            g_mine = dram.tile([1, NNC], f32, tag="gmine")
            g_all = dram.tile([1, N], f32, tag="gall")
            nc.sync.dma_start(out=g_mine[:], in_=g_sb[0:1, 0:NNC])
            nc.gpsimd.collective_compute(
                "AllGather", bass.mybir.AluOpType.bypass,
                replica_groups=[list(range(NCORES))],
                ins=[g_mine[:].opt()], outs=[g_all[:].opt()])

            # interleaved table: table[16G+c, w] = g[16w + c]
            table = tabp.tile([128, TAB_W], f32, tag="table")
            g_all_wc = g_all[0].rearrange("(w c) -> c w", c=16)
            for G in range(8):
                nc.sync.dma_start(out=table[16 * G:16 * G + 16, :], in_=g_all_wc)

            # ---------------- phase 2: edges ----------------
            with tc.tile_pool(name="idx", bufs=3) as idxp, \
                 tc.tile_pool(name="cnd", bufs=3) as cndp, \
                 tc.tile_pool(name="msk", bufs=3) as mskp, \
                 tc.tile_pool(name="edg", bufs=3) as edgp, \
                 tc.tile_pool(name="gate", bufs=3) as gatep, \
                 tc.tile_pool(name="psd", bufs=2, space="PSUM") as psd, \
                 tc.tile_pool(name="pss", bufs=2, space="PSUM") as pss:
                for t in range(NTILES):
                    base = t * TILE_E
                    ps_s = pss.tile([8, M], f32, tag="ps_s")
                    first = True
                    for ed, e_d in ((0, e0_d), (1, e1_d)):
                        # wrapped idx: [128, 32]  <- e[base + 128w + p]
                        ei = idxp.tile([128, M // 16], mybir.dt.int32,
                                       tag=f"ei{ed}")
                        nc.sync.dma_start(
                            out=ei[:],
                            in_=e_d[base:base + TILE_E].rearrange(
                                "(w p) -> p w", p=128))
                        ish = idxp.tile([128, M // 16], mybir.dt.int32,
                                        tag=f"ish{ed}")
                        nc.vector.tensor_scalar(
                            out=ish[:], in0=ei[:], scalar1=4, scalar2=None,
                            op0=mybir.AluOpType.logical_shift_right)
                        iu = idxp.tile([128, M // 16], mybir.dt.uint16,
                                       tag=f"iu{ed}")
                        nc.vector.tensor_copy(out=iu[:], in_=ish[:])
                        cand = cndp.tile([128, M], f32, tag=f"cand{ed}")
                        nc.gpsimd.indirect_copy(
                            out=cand[:], data=table[:], idxs=iu[:],
                            i_know_ap_gather_is_preferred=True)
                        # dense c: [8, M] <- (e[base+16g+128w'+c'] & 15) as f32
                        ec = edgp.tile([8, M], mybir.dt.int32, tag=f"ec{ed}")
                        nc.sync.dma_start(
                            out=ec[:].rearrange("g (w c) -> g w c", c=16),
                            in_=e_d[base:base + TILE_E].rearrange(
                                "(w g c) -> g w c", g=8, c=16))
                        eci = edgp.tile([8, M], mybir.dt.int32, tag=f"eci{ed}")
                        nc.vector.tensor_scalar(
                            out=eci[:], in0=ec[:], scalar1=15, scalar2=None,
                            op0=mybir.AluOpType.bitwise_and)
                        ecf = edgp.tile([8, M], f32, tag=f"ecf{ed}")
                        nc.vector.tensor_copy(out=ecf[:], in_=eci[:])
                        ps_d = psd.tile([128, M], f32, tag=f"psd{ed}")
                        nc.tensor.matmul(out=ps_d[:], lhsT=expand8[:],
                                         rhs=ecf[:], start=True, stop=True)
                        masked = mskp.tile([128, M], f32, tag=f"msk{ed}")
                        nc.vector.scalar_tensor_tensor(
                            out=masked[:], in0=ps_d[:], scalar=iota16[:, 0:1],
                            in1=cand[:], op0=mybir.AluOpType.is_equal,
                            op1=mybir.AluOpType.mult)
                        nc.tensor.matmul(out=ps_s[:], lhsT=bdiag8[:],
                                         rhs=masked[:], start=first,
                                         stop=not first)
                        first = False
                    # gate math on [8, M]
                    nz = edgp.tile([8, M], f32, tag="nz")
                    nc.sync.dma_start(
                        out=nz[:].rearrange("g (w c) -> g w c", c=16),
                        in_=noise_d[base:base + TILE_E].rearrange(
                            "(w g c) -> g w c", g=8, c=16))
                    t1 = gatep.tile([8, M], f32, tag="t1")
                    nc.scalar.activation(out=t1[:], in_=nz[:],
                                         func=mybir.ActivationFunctionType.Ln,
                                         bias=b1t[:, 0:1], scale=a1t[:, 0:1])
                    t2 = gatep.tile([8, M], f32, tag="t2")
                    nc.scalar.activation(out=t2[:], in_=nz[:],
                                         func=mybir.ActivationFunctionType.Ln,
                                         bias=b2t[:, 0:1], scale=a2t[:, 0:1])
                    gt = gatep.tile([8, M], f32, tag="gt")
                    nc.vector.scalar_tensor_tensor(
                        out=gt[:], in0=t1[:], scalar=0.0, in1=t2[:],
                        op0=mybir.AluOpType.add, op1=mybir.AluOpType.subtract)
                    gt2 = gatep.tile([8, M], f32, tag="gt2")
                    nc.vector.scalar_tensor_tensor(
                        out=gt2[:], in0=gt[:], scalar=0.0, in1=ps_s[:],
                        op0=mybir.AluOpType.add, op1=mybir.AluOpType.add)
                    ot = gatep.tile([8, M], f32, tag="ot")
                    nc.scalar.activation(
                        out=ot[:], in_=gt2[:],
                        func=mybir.ActivationFunctionType.Sigmoid)
                    nc.sync.dma_start(
                        out=out_d[base:base + TILE_E].rearrange(
                            "(w g c) -> g w c", g=8, c=16),
                        in_=ot[:].rearrange("g (w c) -> g w c", c=16))
    nc.compile()
    return nc


def _get_runner():
    global _nc, _runner
    if _runner is None:
        from concourse import bass_utils
        _nc = _build()
        _runner = bass_utils
    return _nc, _runner


def kernel(embedding, edges, noise, W_emb, b_emb, W_edge, b_edge):
    nc, bass_utils = _get_runner()
    embedding = np.asarray(embedding, dtype=np.float32)
    edges = np.asarray(edges)
    noise = np.asarray(noise, dtype=np.float32)
    W_emb = np.asarray(W_emb, dtype=np.float32)
    b_emb = np.asarray(b_emb, dtype=np.float32)
    W_edge = np.asarray(W_edge, dtype=np.float32)
    b_edge = np.float32(b_edge)

    wbar = ((W_edge[:HID] + W_edge[HID:]) * 0.5).astype(np.float32)
    wemb = np.ascontiguousarray(W_emb.reshape(2, 128, HID))
    bemb = np.ascontiguousarray(b_emb.reshape(HID, 1))
    wbarr = np.ascontiguousarray(wbar.reshape(HID, 1))
    bhalf = np.array([[b_edge * 0.5]], dtype=np.float32)
    p = np.arange(128)
    expand8 = (p[None, :] // 16 == np.arange(8)[:, None]).astype(np.float32)
    bdiag8 = (p[:, None] // 16 == np.arange(8)[None, :]).astype(np.float32)
    iota16 = (p % 16).astype(np.float32).reshape(128, 1)

    e0 = np.zeros(ECP * NCORES, dtype=np.int32)
    e1 = np.zeros(ECP * NCORES, dtype=np.int32)
    nz = np.full(ECP * NCORES, 0.5, dtype=np.float32)
    for k in range(NCORES):
        e0[k * ECP:k * ECP + EC] = edges[0, k * EC:(k + 1) * EC].astype(np.int32)
        e1[k * ECP:k * ECP + EC] = edges[1, k * EC:(k + 1) * EC].astype(np.int32)
        nz[k * ECP:k * ECP + EC] = noise[k * EC:(k + 1) * EC]

    in_maps = []
    for k in range(NCORES):
        sl = embedding[k * NNC:(k + 1) * NNC]            # [6250, 256]
        embT = np.zeros((IN_DIM, NNCP), dtype=np.float32)
        embT[:, :NNC] = sl.T
        in_maps.append({
            "embT": np.ascontiguousarray(embT.reshape(2, 128, NNCP)),
            "e0": e0[k * ECP:(k + 1) * ECP],
            "e1": e1[k * ECP:(k + 1) * ECP],
            "noise": nz[k * ECP:(k + 1) * ECP],
            "wemb": wemb, "bemb": bemb, "wbar": wbarr, "bhalf": bhalf,
            "expand8": expand8, "bdiag8": bdiag8, "iota16": iota16,
        })

    res = bass_utils.run_bass_kernel_spmd(nc, in_maps,
                                          core_ids=list(range(NCORES)))
    out = np.empty(E, dtype=np.float32)
    for k in range(NCORES):
        out[k * EC:(k + 1) * EC] = res.results[k]["out"][:EC]
    return out


# revision 6
# speedup vs baseline: 1265.6456x; 1265.6456x over previous
"""Trainium2 Bass kernel for nn_Discriminator (GNN edge discriminator).

Algebraic reduction: with w1=W_edge[:H], w2=W_edge[H:],
  s = 0.5*((src@w1+dst@w2) + (dst@w1+src@w2)) = (src+dst) @ wbar,  wbar=(w1+w2)/2
so each node needs only the scalar g[n] = relu(emb[n]@W+b) @ wbar (+ b_edge/2),
and each edge needs s = g[e0]+g[e1]; out = sigmoid(log(eps)-log1p(-eps)+s).

Sharding: nodes row-sharded 6250/core for the g computation, then AllGather;
edges sharded 200K/core. The [50000] g table is held in SBUF 16-way
mod-interleaved (table[p,w] = g[16w+p%16]); per-edge lookups use POOL
indirect_copy (shared column index per 16-partition group = the 16 candidate
values), then a PE broadcast + DVE masked-select + PE block-diagonal reduce
collapse the candidates, accumulating g[e0]+g[e1] straight into PSUM.
"""
import sys
sys.path.insert(0, '/opt/trn_rl_repo')
import numpy as np

N, IN_DIM, HID = 50000, 256, 64
E = 1_600_000
BIAS = 0.0001
NCORES = 8
EC = E // NCORES            # 200000 edges per core
M = 512                     # indices per group per gather instruction
TILE_E = 8 * M              # 4096 edges per pair-tile
NTILES = 49                 # 49*4096 = 200704
ECP = NTILES * TILE_E       # padded edges per core
NNC = N // NCORES           # 6250 nodes per core
NNCP = 6272                 # padded to 49*128 (12*512 + 128)
TAB_W = 3125                # 50000 / 16

_nc = None
_runner = None


def _build():
    from concourse import bass, bacc, tile, mybir

    f32 = mybir.dt.float32
    nc = bacc.Bacc("TRN2", target_bir_lowering=False, debug=False,
                   num_devices=NCORES)

    embT = nc.dram_tensor("embT", [2, 128, NNCP], f32, kind="ExternalInput")
    e0_d = nc.dram_tensor("e0", [ECP], mybir.dt.int32, kind="ExternalInput")
    e1_d = nc.dram_tensor("e1", [ECP], mybir.dt.int32, kind="ExternalInput")
    noise_d = nc.dram_tensor("noise", [ECP], f32, kind="ExternalInput")
    wemb_d = nc.dram_tensor("wemb", [2, 128, HID], f32, kind="ExternalInput")
    bemb_d = nc.dram_tensor("bemb", [HID, 1], f32, kind="ExternalInput")
    wbar_d = nc.dram_tensor("wbar", [HID, 1], f32, kind="ExternalInput")
    bhalf_d = nc.dram_tensor("bhalf", [1, 1], f32, kind="ExternalInput")
    expand8_d = nc.dram_tensor("expand8", [8, 128], f32, kind="ExternalInput")
    bdiag8_d = nc.dram_tensor("bdiag8", [128, 8], f32, kind="ExternalInput")
    iota16_d = nc.dram_tensor("iota16", [128, 1], f32, kind="ExternalInput")
    out_d = nc.dram_tensor("out", [ECP], f32, kind="ExternalOutput")

    a1, b1 = 2.0 * BIAS - 1.0, 1.0 - BIAS     # eps = a1*n + b1
    a2, b2 = 1.0 - 2.0 * BIAS, BIAS           # 1-eps = a2*n + b2

    with tile.TileContext(nc) as tc:
        with tc.tile_pool(name="const", bufs=1) as cp, \
             tc.tile_pool(name="tab", bufs=1) as tabp, \
             tc.tile_pool(name="dram", bufs=1, space="DRAM") as dram:
            w0 = cp.tile([128, HID], f32, tag="w0")
            w1 = cp.tile([128, HID], f32, tag="w1")
            nc.sync.dma_start(out=w0[:], in_=wemb_d[0])
            nc.sync.dma_start(out=w1[:], in_=wemb_d[1])
            bemb = cp.tile([HID, 1], f32, tag="bemb")
            nc.sync.dma_start(out=bemb[:], in_=bemb_d[:, :])
            wbar = cp.tile([HID, 1], f32, tag="wbar")
            nc.sync.dma_start(out=wbar[:], in_=wbar_d[:, :])
            bhalf = cp.tile([1, 1], f32, tag="bhalf")
            nc.sync.dma_start(out=bhalf[:], in_=bhalf_d[:, :])
            expand8 = cp.tile([8, 128], f32, tag="ex8")
            nc.sync.dma_start(out=expand8[:], in_=expand8_d[:, :])
            bdiag8 = cp.tile([128, 8], f32, tag="bd8")
            nc.sync.dma_start(out=bdiag8[:], in_=bdiag8_d[:, :])
            iota16 = cp.tile([128, 1], f32, tag="io16")
            nc.sync.dma_start(out=iota16[:], in_=iota16_d[:, :])
            a1t = cp.tile([8, 1], f32, tag="a1t"); nc.vector.memset(a1t[:], a1)
            b1t = cp.tile([8, 1], f32, tag="b1t"); nc.vector.memset(b1t[:], b1)
            a2t = cp.tile([8, 1], f32, tag="a2t"); nc.vector.memset(a2t[:], a2)
            b2t = cp.tile([8, 1], f32, tag="b2t"); nc.vector.memset(b2t[:], b2)

            # ---------------- phase 1: per-node scalar g ----------------
            g_sb = cp.tile([1, NNCP], f32, tag="gsb")
            with tc.tile_pool(name="p1", bufs=3) as p1, \
                 tc.tile_pool(name="ps1", bufs=2, space="PSUM") as ps1, \
                 tc.tile_pool(name="ps1g", bufs=2, space="PSUM") as ps1g:
                col = 0
                while col < NNCP:
                    n = min(512, NNCP - col)
                    r0 = p1.tile([128, n], f32, tag="r0")
                    r1 = p1.tile([128, n], f32, tag="r1")
                    nc.sync.dma_start(out=r0[:], in_=embT[0, :, col:col + n])
                    nc.sync.dma_start(out=r1[:], in_=embT[1, :, col:col + n])
                    ph = ps1.tile([HID, n], f32, tag="ph")
                    nc.tensor.matmul(out=ph[:], lhsT=w0[:], rhs=r0[:],
                                     start=True, stop=False)
                    nc.tensor.matmul(out=ph[:], lhsT=w1[:], rhs=r1[:],
                                     start=False, stop=True)
                    hT = p1.tile([HID, n], f32, tag="hT")
                    nc.scalar.activation(out=hT[:], in_=ph[:],
                                         func=mybir.ActivationFunctionType.Relu,
                                         bias=bemb[:, 0:1])
                    pg = ps1g.tile([1, n], f32, tag="pg")
                    nc.tensor.matmul(out=pg[:], lhsT=wbar[:], rhs=hT[:],
                                     start=True, stop=True)
                    nc.scalar.activation(out=g_sb[0:1, col:col + n], in_=pg[:],
                                         func=mybir.ActivationFunctionType.Identity,
                                         bias=bhalf[0:1, 0:1])
                    col += n

            g_mine = dram.tile([1, NNC], f32, tag="gmine")
            g_all = dram.tile([1, N], f32, tag="gall")
            nc.sync.dma_start(out=g_mine[:], in_=g_sb[0:1, 0:NNC])
            nc.gpsimd.collective_compute(
                "AllGather", bass.mybir.AluOpType.bypass,
                replica_groups=[list(range(NCORES))],
                ins=[g_mine[:].opt()], outs=[g_all[:].opt()])

            # interleaved table: table[16G+c, w] = g[16w + c]
            table = tabp.tile([128, TAB_W], f32, tag="table")
            g_all_wc = g_all[0].rearrange("(w c) -> c w", c=16)
            for G in range(8):
                nc.sync.dma_start(out=table[16 * G:16 * G + 16, :], in_=g_all_wc)

            # ---------------- phase 2: edges ----------------
            with tc.tile_pool(name="idx", bufs=3) as idxp, \
                 tc.tile_pool(name="cnd", bufs=3) as cndp, \
                 tc.tile_pool(name="msk", bufs=3) as mskp, \
                 tc.tile_pool(name="edg", bufs=3) as edgp, \
                 tc.tile_pool(name="gate", bufs=3) as gatep, \
                 tc.tile_pool(name="psd", bufs=2, space="PSUM") as psd, \
                 tc.tile_pool(name="pss", bufs=2, space="PSUM") as pss:
                for t in range(NTILES):
                    base = t * TILE_E
                    ps_s = pss.tile([8, M], f32, tag="ps_s")
                    first = True
                    for ed, e_d in ((0, e0_d), (1, e1_d)):
                        # wrapped idx: [128, 32]  <- e[base + 128w + p]
                        ei = idxp.tile([128, M // 16], mybir.dt.int32,
                                       tag=f"ei{ed}")
                        nc.sync.dma_start(
                            out=ei[:],
                            in_=e_d[base:base + TILE_E].rearrange(
                                "(w p) -> p w", p=128))
                        ish = idxp.tile([128, M // 16], mybir.dt.int32,
                                        tag=f"ish{ed}")
                        nc.vector.tensor_scalar(
                            out=ish[:], in0=ei[:], scalar1=4, scalar2=None,
                            op0=mybir.AluOpType.logical_shift_right)
                        iu = idxp.tile([128, M // 16], mybir.dt.uint16,
                                       tag=f"iu{ed}")
                        nc.vector.tensor_copy(out=iu[:], in_=ish[:])
                        cand = cndp.tile([128, M], f32, tag=f"cand{ed}")
                        nc.gpsimd.indirect_copy(
                            out=cand[:], data=table[:], idxs=iu[:],
                            i_know_ap_gather_is_preferred=True)
                        # dense c: [8, M] <- (e[base+16g+128w'+c'] & 15) as f32
                        ec = edgp.tile([8, M], mybir.dt.int32, tag=f"ec{ed}")
                        nc.sync.dma_start(
                            out=ec[:].rearrange("g (w c) -> g w c", c=16),
                            in_=e_d[base:base + TILE_E].rearrange(
                                "(w g c) -> g w c", g=8, c=16))
                        eci = edgp.tile([8, M], mybir.dt.int32, tag=f"eci{ed}")
                        nc.vector.tensor_scalar(
                            out=eci[:], in0=ec[:], scalar1=15, scalar2=None,
                            op0=mybir.AluOpType.bitwise_and)
                        ecf = edgp.tile([8, M], f32, tag=f"ecf{ed}")
                        nc.vector.tensor_copy(out=ecf[:], in_=eci[:])
                        ps_d = psd.tile([128, M], f32, tag=f"psd{ed}")
                        nc.tensor.matmul(out=ps_d[:], lhsT=expand8[:],
                                         rhs=ecf[:], start=True, stop=True)
                        masked = mskp.tile([128, M], f32, tag=f"msk{ed}")
                        nc.vector.scalar_tensor_tensor(
                            out=masked[:], in0=ps_d[:], scalar=iota16[:, 0:1],
                            in1=cand[:], op0=mybir.AluOpType.is_equal,
                            op1=mybir.AluOpType.mult)
                        nc.tensor.matmul(out=ps_s[:], lhsT=bdiag8[:],
                                         rhs=masked[:], start=first,
                                         stop=not first)
                        first = False
                    # gate math on [8, M]
                    nz = edgp.tile([8, M], f32, tag="nz")
                    nc.sync.dma_start(
                        out=nz[:].rearrange("g (w c) -> g w c", c=16),
                        in_=noise_d[base:base + TILE_E].rearrange(
                            "(w g c) -> g w c", g=8, c=16))
                    t1 = gatep.tile([8, M], f32, tag="t1")
                    nc.scalar.activation(out=t1[:], in_=nz[:],
                                         func=mybir.ActivationFunctionType.Ln,
                                         bias=b1t[:, 0:1], scale=a1t[:, 0:1])
                    t2 = gatep.tile([8, M], f32, tag="t2")
                    nc.scalar.activation(out=t2[:], in_=nz[:],
                                         func=mybir.ActivationFunctionType.Ln,
                                         bias=b2t[:, 0:1], scale=a2t[:, 0:1])
                    gt = gatep.tile([8, M], f32, tag="gt")
                    nc.vector.scalar_tensor_tensor(
                        out=gt[:], in0=t1[:], scalar=0.0, in1=t2[:],
                        op0=mybir.AluOpType.add, op1=mybir.AluOpType.subtract)
                    gt2 = gatep.tile([8, M], f32, tag="gt2")
                    nc.vector.scalar_tensor_tensor(
                        out=gt2[:], in0=gt[:], scalar=0.0, in1=ps_s[:],
                        op0=mybir.AluOpType.add, op1=mybir.AluOpType.add)
                    ot = gatep.tile([8, M], f32, tag="ot")
                    nc.scalar.activation(
                        out=ot[:], in_=gt2[:],
                        func=mybir.ActivationFunctionType.Sigmoid)
                    nc.sync.dma_start(
                        out=out_d[base:base + TILE_E].rearrange(
                            "(w g c) -> g w c", g=8, c=16),
                        in_=ot[:].rearrange("g (w c) -> g w c", c=16))
    nc.compile()
    return nc


def _get_runner():
    global _nc, _runner
    if _runner is None:
        from concourse import bass_utils
        _nc = _build()
        _runner = bass_utils
    return _nc, _runner


def prepare_in_maps(embedding, edges, noise, W_emb, b_emb, W_edge, b_edge):
    embedding = np.asarray(embedding, dtype=np.float32)
    edges = np.asarray(edges)
    noise = np.asarray(noise, dtype=np.float32)
    W_emb = np.asarray(W_emb, dtype=np.float32)
    b_emb = np.asarray(b_emb, dtype=np.float32)
    W_edge = np.asarray(W_edge, dtype=np.float32)
    b_edge = np.float32(b_edge)

    wbar = ((W_edge[:HID] + W_edge[HID:]) * 0.5).astype(np.float32)
    wemb = np.ascontiguousarray(W_emb.reshape(2, 128, HID))
    bemb = np.ascontiguousarray(b_emb.reshape(HID, 1))
    wbarr = np.ascontiguousarray(wbar.reshape(HID, 1))
    bhalf = np.array([[b_edge * 0.5]], dtype=np.float32)
    p = np.arange(128)
    expand8 = (p[None, :] // 16 == np.arange(8)[:, None]).astype(np.float32)
    bdiag8 = (p[:, None] // 16 == np.arange(8)[None, :]).astype(np.float32)
    iota16 = (p % 16).astype(np.float32).reshape(128, 1)

    e0 = np.zeros(ECP * NCORES, dtype=np.int32)
    e1 = np.zeros(ECP * NCORES, dtype=np.int32)
    nz = np.full(ECP * NCORES, 0.5, dtype=np.float32)
    for k in range(NCORES):
        e0[k * ECP:k * ECP + EC] = edges[0, k * EC:(k + 1) * EC].astype(np.int32)
        e1[k * ECP:k * ECP + EC] = edges[1, k * EC:(k + 1) * EC].astype(np.int32)
        nz[k * ECP:k * ECP + EC] = noise[k * EC:(k + 1) * EC]

    in_maps = []
    for k in range(NCORES):
        sl = embedding[k * NNC:(k + 1) * NNC]            # [6250, 256]
        embT = np.zeros((IN_DIM, NNCP), dtype=np.float32)
        embT[:, :NNC] = sl.T
        in_maps.append({
            "embT": np.ascontiguousarray(embT.reshape(2, 128, NNCP)),
            "e0": e0[k * ECP:(k + 1) * ECP],
            "e1": e1[k * ECP:(k + 1) * ECP],
            "noise": nz[k * ECP:(k + 1) * ECP],
            "wemb": wemb, "bemb": bemb, "wbar": wbarr, "bhalf": bhalf,
            "expand8": expand8, "bdiag8": bdiag8, "iota16": iota16,
        })

    return in_maps


def kernel(embedding, edges, noise, W_emb, b_emb, W_edge, b_edge):
    nc, bass_utils = _get_runner()
    in_maps = prepare_in_maps(embedding, edges, noise, W_emb, b_emb,
                              W_edge, b_edge)
    res = bass_utils.run_bass_kernel_spmd(nc, in_maps,
                                          core_ids=list(range(NCORES)))
    out = np.empty(E, dtype=np.float32)
    for k in range(NCORES):
        out[k * EC:(k + 1) * EC] = res.results[k]["out"][:EC]
    return out
